# revision 26
# baseline (speedup 1.0000x reference)
"""Trainium2 Bass kernel for GQA attention block (B=2, S=2048, HID=4096, 32Q/8KV heads).

Sharding: hybrid TP4 x DP2 over 8 NeuronCores.
  core c: batch b = c // 4, TP slice t = c % 4.
  Each core: one batch element, 8 Q heads (2 KV heads); o_proj partials summed on host.

v2 structure (PE-bound by design; bf16 everywhere):
  Phase 1  K proj (feature-major) + V proj (token-major, no PE transposes), streamed
           over 4 token blocks in order [1,2,3,0] so the last-loaded hs tiles are
           step 0's, reused by phase 2.
  Phase 2  merged Q-proj + attention pipeline over 32 (token-step, head) units.
           Attention for unit j interleaves Q-proj matmuls of unit j+2 into the PE
           stream, filling the exp-latency gaps; scalar exp spreads over the whole
           phase instead of pacing it. Scores kt-pairs run 2 ahead of attn*V pairs.
  Phase 3  output projection (partial; host sums over TP).
Softmax: no row max (per reference); denominator via bf16 add-tree + ones-matmul
partition reduction; RoPE rotate_half = two partition-shift DMAs w/ sign folded
into the host-negated sin table.
"""
import os
import sys

for _p in ("/opt/trn_rl_repo", "/root/.axon_site"):
    if _p not in sys.path and os.path.isdir(_p):
        sys.path.append(_p)

import numpy as np

B, S_FULL, HID = 2, 2048, 4096
NH, NKV, HD = 32, 8, 128
TP = 4                 # tensor-parallel ways
QH = NH // TP          # 8 q heads per core
KVH = NKV // TP        # 2 kv heads per core
FQ = QH * HD           # 1024
FKV = KVH * HD         # 256
KH = HID // 128        # 32 contraction tiles
SCALE = 1.0 / float(np.sqrt(HD))

last_exec_time_ns = None


def build_nc(S: int = S_FULL, dt: str = "bf16"):
    """Build the per-core Bass program (SPMD: same program, per-core inputs)."""
    import concourse.bass as bass
    import concourse.tile as tile
    from concourse import bacc, bass_isa, mybir
    from contextlib import ExitStack

    f32 = mybir.dt.float32
    mdt = mybir.dt.bfloat16
    TBQ = 512                      # token block (= attention query block)
    NSTEP = S // TBQ               # 4
    KT = S // 128                  # 16 key/token tiles
    KTB = TBQ // 128               # 4 token tiles per block
    NPAIR = KT // 2                # 8 score kt-pairs per attention unit
    HALF = HD // 2

    nc = bacc.Bacc("TRN2", target_bir_lowering=False, debug=False)

    hsT = nc.dram_tensor("hsT", [HID, S], mdt, kind="ExternalInput")
    cosT = nc.dram_tensor("cosT", [HD, S], mdt, kind="ExternalInput")
    sinT = nc.dram_tensor("sinT", [HD, S], mdt, kind="ExternalInput")  # sign-folded
    wqh = nc.dram_tensor("wqh", [QH * HID, HD], mdt, kind="ExternalInput")  # head-major
    bq = nc.dram_tensor("bq", [QH, HD], f32, kind="ExternalInput")
    wkv = nc.dram_tensor("wkv", [HID, 2 * FKV], mdt, kind="ExternalInput")
    bk = nc.dram_tensor("bk", [KVH, HD], f32, kind="ExternalInput")
    bvb = nc.dram_tensor("bvb", [128, FKV], f32, kind="ExternalInput")  # pre-broadcast
    wo = nc.dram_tensor("wo", [FQ, HID], mdt, kind="ExternalInput")
    ones = nc.dram_tensor("ones", [128, 128], mdt, kind="ExternalInput")
    out = nc.dram_tensor("out", [S, HID], f32, kind="ExternalOutput")

    with tile.TileContext(nc) as tc, ExitStack() as ctx:
        Exp = mybir.ActivationFunctionType.Exp
        Ident = mybir.ActivationFunctionType.Identity

        const = ctx.enter_context(tc.tile_pool(name="const", bufs=1))
        bq_t = const.tile([128, QH], f32)
        bk_t = const.tile([128, KVH], f32)
        bvb_t = const.tile([128, FKV], f32)
        ones_t = const.tile([128, 128], mdt)
        cos_t = const.tile([128, S], mdt)
        sin_t = const.tile([128, S], mdt)
        dummy = const.tile([128, 1], f32)
        # pre-trigger the exp table-set load with no data dependencies
        nc.scalar.activation(dummy[:], dummy[:], mybir.ActivationFunctionType.Exp)

        # Persistent activations (feature-major). attn output overwrites q in place.
        qpool = ctx.enter_context(tc.tile_pool(name="qpool", bufs=1))
        q_t = [qpool.tile([128, S], mdt, name=f"q{h}") for h in range(QH)]
        kvpool = ctx.enter_context(tc.tile_pool(name="kvpool", bufs=1))
        k_t = [kvpool.tile([128, S], mdt, name=f"k{f}") for f in range(KVH)]
        v_t = kvpool.tile([128, KT * FKV], mdt, name="v")  # [tok%128, (kt, kv*128+d)]

        rope_pool = ctx.enter_context(tc.tile_pool(name="ropep", bufs=1))

        def rope_inplace(x_t, sl, tag):
            """x[:, sl] = x[:, sl]*cos + shift64(x[:, sl])*sin' (sign in sin')."""
            w = sl.stop - sl.start
            rot = rope_pool.tile([128, TBQ], mdt, name=f"rot_{tag}", tag="rot", bufs=3)
            nc.sync.dma_start(rot[0:HALF, :w], x_t[HALF:128, sl])
            nc.sync.dma_start(rot[HALF:128, :w], x_t[0:HALF, sl])
            t1 = rope_pool.tile([128, TBQ], mdt, name=f"t1_{tag}", tag="t1", bufs=3)
            nc.vector.tensor_mul(t1[:, :w], rot[:, :w], sin_t[:, sl])
            nc.vector.tensor_mul(x_t[:, sl], x_t[:, sl], cos_t[:, sl])
            nc.vector.tensor_add(x_t[:, sl], x_t[:, sl], t1[:, :w])

        # Shared hs tile store: [128, TBQ] per (tb, k); KV streams them in, QA
        # (Q projection) re-reads step tiles. bufs=64 = two full steps resident.
        hsp = ctx.enter_context(tc.tile_pool(name="hsp", bufs=1))
        hs_tiles = {}

        def load_hs(tb, k):
            t = hsp.tile([128, TBQ], mdt, name=f"hs_{tb}_{k}", tag="hs", bufs=64)
            nc.sync.dma_start(t[:], hsT.ap()[k * 128:(k + 1) * 128,
                                             tb * TBQ:(tb + 1) * TBQ])
            hs_tiles[(tb, k)] = t
            return t

        # Q weights, head-major streamed: one [128, KH*HD] tile per (tb, h) unit.
        qwp = ctx.enter_context(tc.tile_pool(name="qwp", bufs=1))
        wq_tiles = {}

        def load_wq(j):
            tb, h = UNITS[j]
            t = qwp.tile([128, KH * HD], mdt, name=f"wq_{j}", tag="wq", bufs=3)
            for c in range(4):
                nc.sync.dma_start(
                    t[:, c * 8 * HD:(c + 1) * 8 * HD]
                    .rearrange("p (k d) -> p k d", k=8),
                    wqh.ap()[h * HID + c * 1024:h * HID + (c + 1) * 1024, :]
                    .rearrange("(k p) d -> p k d", p=128))
            wq_tiles[j] = t

        UNITS = [(tb, h) for tb in range(NSTEP) for h in range(QH)]

        # Q-proj accumulators live in the outer scope so warmup chunks can be
        # interleaved into the last KV block (2 PSUM banks, +6 for pkv = 8).
        psqp = ctx.enter_context(tc.tile_pool(name="psq", bufs=1, space="PSUM"))
        qst = {}   # j -> psq tile
        qci = {}   # j -> next chunk index

        def q_chunk(j, ci):
            """Emit 4 Q-proj matmuls (k = 4ci..4ci+3) for unit j."""
            tb, h = UNITS[j]
            if ci == 0:
                qst[j] = psqp.tile([128, TBQ], f32, name=f"psq_{j}",
                                   tag="psq", bufs=2)
            wq_t = wq_tiles[j]
            ps = qst[j]
            for k in range(4 * ci, 4 * ci + 4):
                nc.tensor.matmul(ps[:], wq_t[:, k * 128:(k + 1) * 128],
                                 hs_tiles[(tb, k)][:],
                                 start=(k == 0), stop=(k == KH - 1))

        def emit_q(j, n=1):
            c0 = qci.get(j, 0)
            for ci in range(c0, min(c0 + n, 8)):
                q_chunk(j, ci)
            qci[j] = min(c0 + n, 8)

        def q_finish(j):
            """Evacuate + bias + rope unit j's q block."""
            tb, h = UNITS[j]
            sl = slice(tb * TBQ, (tb + 1) * TBQ)
            nc.scalar.activation(q_t[h][:, sl], qst.pop(j)[:], Ident,
                                 bias=bq_t[:, h:h + 1])
            rope_inplace(q_t[h], sl, f"q_{j}")

        # ---- Phase 1: K proj (feature-major) + V proj (token-major) ----
        kv_order = list(range(1, NSTEP)) + [0]
        with (
            tc.tile_pool(name="kvw", bufs=1) as kvw,
            tc.tile_pool(name="pkv", bufs=1, space="PSUM") as pkv,
        ):
            wkv_k = []
            for ti, tb in enumerate(kv_order):
                tb0 = tb * TBQ
                psk = [pkv.tile([128, TBQ], f32, name=f"psk_{tb}_{f}",
                                tag=f"pk{f}", bufs=2) for f in range(KVH)]
                psv = pkv.tile([128, KTB * FKV], f32, name=f"psv_{tb}",
                               tag="pv", bufs=1)
                for k in range(KH):
                    if ti == 2 and k == 16:
                        load_wq(2)
                    hs_s = load_hs(tb, k)
                    if ti == 0:
                        w = kvw.tile([128, 2 * FKV], mdt, name=f"wkv_{k}")
                        nc.scalar.dma_start(w[:], wkv.ap()[k * 128:(k + 1) * 128, :])
                        wkv_k.append(w)
                        if k == 0:
                            nc.scalar.dma_start(bq_t[:], bq.ap().rearrange("h p -> p h"))
                            nc.scalar.dma_start(bk_t[:], bk.ap().rearrange("h p -> p h"))
                            nc.scalar.dma_start(bvb_t[:], bvb.ap())
                            nc.scalar.dma_start(ones_t[:], ones.ap())
                        if k == 8:
                            nc.scalar.dma_start(cos_t[:], cosT.ap())
                            nc.scalar.dma_start(sin_t[:], sinT.ap())
                    w = wkv_k[k]

                    def k_mm(f):
                        nc.tensor.matmul(psk[f][:], w[:, f * 128:(f + 1) * 128],
                                         hs_s[:], start=(k == 0), stop=(k == KH - 1))

                    def v_mm(tt):
                        # psv packs two 1KB accum slices per 2KB PSUM bank;
                        # start=True zeroes the whole bank (zero-region), so
                        # only the first slice of each bank may issue it.
                        nc.tensor.matmul(psv[:, tt * FKV:(tt + 1) * FKV],
                                         hs_s[:, tt * 128:(tt + 1) * 128],
                                         w[:, FKV:2 * FKV],
                                         start=(k == 0 and tt % 2 == 0),
                                         stop=(k == KH - 1))

                    # interleave so consecutive matmuls hit different banks
                    k_mm(0); v_mm(0); v_mm(2); k_mm(1); v_mm(1); v_mm(3)
                    # warmup: Q proj chunks for units 0/1 ride the last block
                    if ti == NSTEP - 1 and k % 4 == 3:
                        emit_q(0, 1)
                        emit_q(1, 1)
                # evacuations (v first: shortens the vector tail ahead of the
                # q-rope chain at the phase boundary) + K rope
                for tt in range(KTB):
                    kt = tb * KTB + tt
                    nc.vector.tensor_add(v_t[:, kt * FKV:(kt + 1) * FKV],
                                         psv[:, tt * FKV:(tt + 1) * FKV], bvb_t[:])
                for f in range(KVH):
                    nc.scalar.activation(k_t[f][:, tb0:tb0 + TBQ], psk[f][:], Ident,
                                         bias=bk_t[:, f:f + 1])
                for f in range(KVH):
                    rope_inplace(k_t[f], slice(tb0, tb0 + TBQ), f"k{f}_{tb}")
                if ti == 1:
                    load_wq(0)
                elif ti == 2:
                    load_wq(1)

        # ---- Phase 2: merged Q-proj + attention pipeline ----
        st3 = ctx.enter_context(tc.tile_pool(name="st3", bufs=4))  # wo prefetch
        wo_pre = []
        with (
            tc.tile_pool(name="expp", bufs=1) as expp,
            tc.tile_pool(name="spool", bufs=1) as spool,
            tc.tile_pool(name="invp", bufs=2) as invp,
            tc.tile_pool(name="pss", bufs=1, space="PSUM") as pss,
            tc.tile_pool(name="pso", bufs=1, space="PSUM") as pso,
        ):
            q_finish(0)
            q_finish(1)
            load_wq(3)

            for j, (tb, h) in enumerate(UNITS):
                sl = slice(tb * TBQ, (tb + 1) * TBQ)
                f = h // (QH // KVH)
                qj = j + 2 if j + 2 < len(UNITS) else None
                if j + 4 < len(UNITS):
                    load_wq(j + 4)
                # prefetch next step's hs tiles (8 per unit over h = 3..6)
                if 3 <= h <= 6 and tb + 1 < NSTEP:
                    for k in range((h - 3) * 8, (h - 3) * 8 + 8):
                        load_hs(tb + 1, k)
                if j == len(UNITS) - 3:
                    for fh in range(QH):
                        w = st3.tile([128, 512], mdt, name=f"wo_0_{fh}",
                                     tag=f"wo{fh}", bufs=2)
                        nc.sync.dma_start(w[:], wo.ap()[fh * 128:(fh + 1) * 128, 0:512])
                        wo_pre.append(w)

                po = pso.tile([128, TBQ], f32, name=f"po_{j}", tag="oo", bufs=2)
                ets = []

                def s_pair(p):
                    ps = pss.tile([128, 2 * TBQ], f32, name=f"ps_{j}_{p}",
                                  tag="ss", bufs=2)
                    for i2 in range(2):
                        kt = 2 * p + i2
                        nc.tensor.matmul(ps[:, i2 * TBQ:(i2 + 1) * TBQ],
                                         k_t[f][:, kt * 128:(kt + 1) * 128],
                                         q_t[h][:, sl], start=True, stop=True)
                    et = expp.tile([128, 2 * TBQ], mdt, name=f"e_{j}_{p}",
                                   tag="et", bufs=5)
                    nc.scalar.activation(et[:], ps[:], Exp, scale=SCALE)
                    ets.append(et)

                def a_pair(idx, p):
                    et = ets[idx]
                    for i2 in range(2):
                        kt = 2 * p + i2
                        nc.tensor.matmul(po[:],
                                         v_t[:, kt * FKV + f * 128:
                                             kt * FKV + (f + 1) * 128],
                                         et[:, i2 * TBQ:(i2 + 1) * TBQ],
                                         start=(idx == 0 and i2 == 0),
                                         stop=(idx == NPAIR - 1 and i2 == 1))

                # leading Q chunks have no dependency on this unit's rope, so
                # they lead the unit and absorb scalar/vector tail latency
                # process kt-pairs of the last-roped block (tb=0, pairs 0,1)
                # last so unit 0 never waits on the freshest K rope
                porder = [2, 3, 4, 5, 6, 7, 0, 1][:NPAIR]
                # last two units have no Q-proj left; fill their PE slack with
                # the first o-proj tiles (hb=0, tt=0..3) instead
                eo_tts = []
                if qj is None and NSTEP * QH - j <= 2:
                    eo_tts = [0, 1] if j == len(UNITS) - 2 else [2, 3]
                eo_pots = {}
                if qj is not None:
                    emit_q(qj, 5 if j == 0 else 1)
                s_pair(porder[0])
                s_pair(porder[1])
                t1s = []
                for i in range(NPAIR):
                    if qj is not None:
                        emit_q(qj, 1)
                    for tt in eo_tts:
                        if i == 0:
                            eo_pots[tt] = psqp.tile([128, 512], f32,
                                                    name=f"eo_{tt}", tag="psq",
                                                    bufs=2)
                        nc.tensor.matmul(eo_pots[tt][:],
                                         q_t[i][:, tt * 128:(tt + 1) * 128],
                                         wo_pre[i][:],
                                         start=(i == 0), stop=(i == QH - 1))
                    if i + 2 < NPAIR:
                        s_pair(porder[i + 2])
                    a_pair(i, porder[i])
                    if i % 2 == 1:
                        m = (i - 1) // 2
                        t1 = spool.tile([128, 2 * TBQ], mdt, name=f"t1_{j}_{m}",
                                        tag="pa", bufs=5)
                        nc.vector.tensor_add(t1[:], ets[i - 1][:], ets[i][:])
                        t1s.append(t1)
                u0 = spool.tile([128, 2 * TBQ], mdt, name=f"u0_{j}", tag="u2", bufs=2)
                nc.vector.tensor_add(u0[:], t1s[0][:], t1s[1][:])
                u1 = spool.tile([128, 2 * TBQ], mdt, name=f"u1_{j}", tag="u3", bufs=2)
                nc.vector.tensor_add(u1[:], t1s[2][:], t1s[3][:])
                nc.vector.tensor_add(u0[:], u0[:], u1[:])
                ssum = spool.tile([128, TBQ], mdt, name=f"sm_{j}", tag="sm", bufs=2)
                nc.vector.tensor_add(ssum[:], u0[:, 0:TBQ], u0[:, TBQ:2 * TBQ])
                pb = spool.tile([128, TBQ], f32, name=f"pb_{j}", tag="pbs", bufs=1)
                nc.gpsimd.partition_all_reduce(pb[:], ssum[:], 128,
                                               bass_isa.ReduceOp.add)
                inv = invp.tile([128, TBQ], f32, name=f"inv_{j}", tag="inv")
                nc.vector.reciprocal_approx_fast(inv[:], pb[:])
                # normalized attn output, overwrites q head block in place
                nc.vector.tensor_mul(q_t[h][:, sl], po[:], inv[:])
                for tt in eo_tts:
                    ot = invp.tile([128, 512], f32, name=f"eot_{tt}", tag="ot0",
                                   bufs=2)
                    nc.scalar.copy(ot[:], eo_pots[tt][:])
                    nc.sync.dma_start(out.ap()[tt * 128:(tt + 1) * 128, 0:512],
                                      ot[:])
                if qj is not None:
                    emit_q(qj, 8)
                    q_finish(qj)

        # ---- Phase 3: output projection (partial; host sums over TP) ----
        with (
            tc.tile_pool(name="osb", bufs=4) as osb,
            tc.tile_pool(name="po5", bufs=1, space="PSUM") as po5,
        ):
            NHB = HID // 512
            for hb in range(NHB):
                if hb == 0:
                    wo_s = wo_pre
                else:
                    wo_s = []
                    for fh in range(QH):
                        w = st3.tile([128, 512], mdt, name=f"wo_{hb}_{fh}",
                                     tag=f"wo{fh}", bufs=2)
                        nc.sync.dma_start(
                            w[:], wo.ap()[fh * 128:(fh + 1) * 128,
                                          hb * 512:(hb + 1) * 512])
                        wo_s.append(w)
                for tt in range(4 if hb == 0 else 0, KT):
                    pot = po5.tile([128, 512], f32, name=f"pot_{hb}_{tt}",
                                   tag="po", bufs=4)
                    for fh in range(QH):
                        nc.tensor.matmul(pot[:], q_t[fh][:, tt * 128:(tt + 1) * 128],
                                         wo_s[fh][:], start=(fh == 0),
                                         stop=(fh == QH - 1))
                    ot = osb.tile([128, 512], f32, name=f"ot_{hb}_{tt}", tag="ot")
                    nc.scalar.copy(ot[:], pot[:])
                    nc.sync.dma_start(
                        out.ap()[tt * 128:(tt + 1) * 128, hb * 512:(hb + 1) * 512],
                        ot[:])

    nc.compile()
    return nc


def shard_inputs(hidden_states, cos, sin, Wq, bq, Wk, bk, Wv, bv, Wo, S=S_FULL,
                 dt="bf16"):
    import ml_dtypes
    big = ml_dtypes.bfloat16
    ones = np.ones((128, 128), dtype=np.float32)
    in_maps = []
    for c in range(8):
        b, t = c // TP, c % TP
        sinT = np.ascontiguousarray(sin[b].T).astype(np.float32)
        sinT[:HD // 2, :] *= -1.0   # rotate_half sign folded into the table
        wq_slice = Wq[:, t * FQ:(t + 1) * FQ]                      # [HID, FQ]
        wqh = np.ascontiguousarray(
            wq_slice.reshape(HID, QH, HD).transpose(1, 0, 2).reshape(QH * HID, HD))
        bv_slice = bv[t * FKV:(t + 1) * FKV].astype(np.float32)
        m = {
            "hsT": np.ascontiguousarray(hidden_states[b].T).astype(big),
            "cosT": np.ascontiguousarray(cos[b].T).astype(big),
            "sinT": sinT.astype(big),
            "wqh": wqh.astype(big),
            "bq": np.ascontiguousarray(bq[t * FQ:(t + 1) * FQ].reshape(QH, HD)),
            "bk": np.ascontiguousarray(bk[t * FKV:(t + 1) * FKV].reshape(KVH, HD)),
            "bvb": np.ascontiguousarray(
                np.broadcast_to(bv_slice[None, :], (128, FKV))).astype(np.float32),
            "wkv": np.ascontiguousarray(np.concatenate(
                [Wk[:, t * FKV:(t + 1) * FKV], Wv[:, t * FKV:(t + 1) * FKV]],
                axis=1)).astype(big),
            "wo": np.ascontiguousarray(Wo[t * FQ:(t + 1) * FQ, :]).astype(big),
            "ones": ones.astype(big),
        }
        in_maps.append(m)
    return in_maps


_nc_cache = {}


def kernel(hidden_states, cos, sin, Wq, bq, Wk, bk, Wv, bv, Wo):
    global last_exec_time_ns
    from concourse.bass_utils import run_bass_kernel_spmd

    hidden_states = np.asarray(hidden_states, dtype=np.float32)
    cos = np.asarray(cos, dtype=np.float32)
    sin = np.asarray(sin, dtype=np.float32)
    S = hidden_states.shape[1]
    dt = "bf16"
    if (S, dt) not in _nc_cache:
        _nc_cache[(S, dt)] = build_nc(S, dt)
    nc = _nc_cache[(S, dt)]
    in_maps = shard_inputs(hidden_states, cos, sin,
                           np.asarray(Wq, np.float32), np.asarray(bq, np.float32),
                           np.asarray(Wk, np.float32), np.asarray(bk, np.float32),
                           np.asarray(Wv, np.float32), np.asarray(bv, np.float32),
                           np.asarray(Wo, np.float32), S=S, dt=dt)
    trace = bool(int(os.environ.get("ATTN_TRACE", "0")))
    r = run_bass_kernel_spmd(nc, in_maps, list(range(8)), trace=trace)
    last_exec_time_ns = r.exec_time_ns
    outs = [r.results[c]["out"] for c in range(8)]
    full = np.empty((B, S, HID), dtype=np.float32)
    for b in range(B):
        full[b] = outs[b * TP]
        for t in range(1, TP):
            full[b] += outs[b * TP + t]
    return full


# revision 28
# speedup vs baseline: 1.0050x; 1.0050x over previous
"""Trainium2 Bass kernel for GQA attention block (B=2, S=2048, HID=4096, 32Q/8KV heads).

Sharding: hybrid TP4 x DP2 over 8 NeuronCores.
  core c: batch b = c // 4, TP slice t = c % 4.
  Each core: one batch element, 8 Q heads (2 KV heads); o_proj partials summed on host.

v2 structure (PE-bound by design; bf16 everywhere):
  Phase 1  K proj (feature-major) + V proj (token-major, no PE transposes), streamed
           over 4 token blocks in order [1,2,3,0] so the last-loaded hs tiles are
           step 0's, reused by phase 2.
  Phase 2  merged Q-proj + attention pipeline over 32 (token-step, head) units.
           Attention for unit j interleaves Q-proj matmuls of unit j+2 into the PE
           stream, filling the exp-latency gaps; scalar exp spreads over the whole
           phase instead of pacing it. Scores kt-pairs run 2 ahead of attn*V pairs.
  Phase 3  output projection (partial; host sums over TP).
Softmax: no row max (per reference); denominator via bf16 add-tree + ones-matmul
partition reduction; RoPE rotate_half = two partition-shift DMAs w/ sign folded
into the host-negated sin table.
"""
import os
import sys

for _p in ("/opt/trn_rl_repo", "/root/.axon_site"):
    if _p not in sys.path and os.path.isdir(_p):
        sys.path.append(_p)

import numpy as np

B, S_FULL, HID = 2, 2048, 4096
NH, NKV, HD = 32, 8, 128
TP = 4                 # tensor-parallel ways
QH = NH // TP          # 8 q heads per core
KVH = NKV // TP        # 2 kv heads per core
FQ = QH * HD           # 1024
FKV = KVH * HD         # 256
KH = HID // 128        # 32 contraction tiles
SCALE = 1.0 / float(np.sqrt(HD))

last_exec_time_ns = None


def build_nc(S: int = S_FULL, dt: str = "bf16"):
    """Build the per-core Bass program (SPMD: same program, per-core inputs)."""
    import concourse.bass as bass
    import concourse.tile as tile
    from concourse import bacc, bass_isa, mybir
    from contextlib import ExitStack

    f32 = mybir.dt.float32
    mdt = mybir.dt.bfloat16
    TBQ = 512                      # token block (= attention query block)
    NSTEP = S // TBQ               # 4
    KT = S // 128                  # 16 key/token tiles
    KTB = TBQ // 128               # 4 token tiles per block
    NPAIR = KT // 2                # 8 score kt-pairs per attention unit
    HALF = HD // 2

    nc = bacc.Bacc("TRN2", target_bir_lowering=False, debug=False)

    hsT = nc.dram_tensor("hsT", [HID, S], mdt, kind="ExternalInput")
    cosT = nc.dram_tensor("cosT", [HD, S], mdt, kind="ExternalInput")
    sinT = nc.dram_tensor("sinT", [HD, S], mdt, kind="ExternalInput")  # sign-folded
    wqh = nc.dram_tensor("wqh", [QH * HID, HD], mdt, kind="ExternalInput")  # head-major
    bq = nc.dram_tensor("bq", [QH, HD], f32, kind="ExternalInput")
    wkv = nc.dram_tensor("wkv", [HID, 2 * FKV], mdt, kind="ExternalInput")
    bk = nc.dram_tensor("bk", [KVH, HD], f32, kind="ExternalInput")
    bvb = nc.dram_tensor("bvb", [128, FKV], f32, kind="ExternalInput")  # pre-broadcast
    wo = nc.dram_tensor("wo", [FQ, HID], mdt, kind="ExternalInput")
    ones = nc.dram_tensor("ones", [128, 128], mdt, kind="ExternalInput")
    out = nc.dram_tensor("out", [S, HID], f32, kind="ExternalOutput")

    with tile.TileContext(nc) as tc, ExitStack() as ctx:
        Exp = mybir.ActivationFunctionType.Exp
        Ident = mybir.ActivationFunctionType.Identity

        const = ctx.enter_context(tc.tile_pool(name="const", bufs=1))
        bq_t = const.tile([128, QH], f32)
        bk_t = const.tile([128, KVH], f32)
        bvb_t = const.tile([128, FKV], f32)
        ones_t = const.tile([128, 128], mdt)
        cos_t = const.tile([128, S], mdt)
        sin_t = const.tile([128, S], mdt)
        dummy = const.tile([128, 1], f32)
        # pre-trigger the exp table-set load with no data dependencies
        nc.scalar.activation(dummy[:], dummy[:], mybir.ActivationFunctionType.Exp)

        # Persistent activations (feature-major). attn output overwrites q in place.
        qpool = ctx.enter_context(tc.tile_pool(name="qpool", bufs=1))
        q_t = [qpool.tile([128, S], mdt, name=f"q{h}") for h in range(QH)]
        kvpool = ctx.enter_context(tc.tile_pool(name="kvpool", bufs=1))
        k_t = [kvpool.tile([128, S], mdt, name=f"k{f}") for f in range(KVH)]
        v_t = kvpool.tile([128, KT * FKV], mdt, name="v")  # [tok%128, (kt, kv*128+d)]

        rope_pool = ctx.enter_context(tc.tile_pool(name="ropep", bufs=1))

        def rope_inplace(x_t, sl, tag):
            """x[:, sl] = x[:, sl]*cos + shift64(x[:, sl])*sin' (sign in sin')."""
            w = sl.stop - sl.start
            rot = rope_pool.tile([128, TBQ], mdt, name=f"rot_{tag}", tag="rot", bufs=3)
            nc.sync.dma_start(rot[0:HALF, :w], x_t[HALF:128, sl])
            nc.sync.dma_start(rot[HALF:128, :w], x_t[0:HALF, sl])
            t1 = rope_pool.tile([128, TBQ], mdt, name=f"t1_{tag}", tag="t1", bufs=3)
            nc.vector.tensor_mul(t1[:, :w], rot[:, :w], sin_t[:, sl])
            nc.vector.tensor_mul(x_t[:, sl], x_t[:, sl], cos_t[:, sl])
            nc.vector.tensor_add(x_t[:, sl], x_t[:, sl], t1[:, :w])

        # Shared hs tile store: [128, TBQ] per (tb, k); KV streams them in, QA
        # (Q projection) re-reads step tiles. bufs=64 = two full steps resident.
        hsp = ctx.enter_context(tc.tile_pool(name="hsp", bufs=1))
        hs_tiles = {}

        def load_hs(tb, k):
            t = hsp.tile([128, TBQ], mdt, name=f"hs_{tb}_{k}", tag="hs", bufs=64)
            nc.sync.dma_start(t[:], hsT.ap()[k * 128:(k + 1) * 128,
                                             tb * TBQ:(tb + 1) * TBQ])
            hs_tiles[(tb, k)] = t
            return t

        # Q weights, head-major streamed: one [128, KH*HD] tile per (tb, h) unit.
        qwp = ctx.enter_context(tc.tile_pool(name="qwp", bufs=1))
        wq_tiles = {}

        def load_wq(j):
            tb, h = UNITS[j]
            t = qwp.tile([128, KH * HD], mdt, name=f"wq_{j}", tag="wq", bufs=3)
            for c in range(4):
                nc.sync.dma_start(
                    t[:, c * 8 * HD:(c + 1) * 8 * HD]
                    .rearrange("p (k d) -> p k d", k=8),
                    wqh.ap()[h * HID + c * 1024:h * HID + (c + 1) * 1024, :]
                    .rearrange("(k p) d -> p k d", p=128))
            wq_tiles[j] = t

        UNITS = [(tb, h) for tb in range(NSTEP) for h in range(QH)]

        # Q-proj accumulators live in the outer scope so warmup chunks can be
        # interleaved into the last KV block (2 PSUM banks, +6 for pkv = 8).
        psqp = ctx.enter_context(tc.tile_pool(name="psq", bufs=1, space="PSUM"))
        qst = {}   # j -> psq tile
        qci = {}   # j -> next chunk index

        def q_chunk(j, ci):
            """Emit 4 Q-proj matmuls (k = 4ci..4ci+3) for unit j."""
            tb, h = UNITS[j]
            if ci == 0:
                qst[j] = psqp.tile([128, TBQ], f32, name=f"psq_{j}",
                                   tag="psq", bufs=2)
            wq_t = wq_tiles[j]
            ps = qst[j]
            for k in range(4 * ci, 4 * ci + 4):
                nc.tensor.matmul(ps[:], wq_t[:, k * 128:(k + 1) * 128],
                                 hs_tiles[(tb, k)][:],
                                 start=(k == 0), stop=(k == KH - 1))

        def emit_q(j, n=1):
            c0 = qci.get(j, 0)
            for ci in range(c0, min(c0 + n, 8)):
                q_chunk(j, ci)
            qci[j] = min(c0 + n, 8)

        def q_finish(j):
            """Evacuate + bias + rope unit j's q block."""
            tb, h = UNITS[j]
            sl = slice(tb * TBQ, (tb + 1) * TBQ)
            nc.scalar.activation(q_t[h][:, sl], qst.pop(j)[:], Ident,
                                 bias=bq_t[:, h:h + 1])
            rope_inplace(q_t[h], sl, f"q_{j}")

        # ---- Phase 1: K proj (feature-major) + V proj (token-major) ----
        kv_order = list(range(1, NSTEP)) + [0]
        with (
            tc.tile_pool(name="kvw", bufs=1) as kvw,
            tc.tile_pool(name="pkv", bufs=1, space="PSUM") as pkv,
        ):
            wkv_k = []
            for ti, tb in enumerate(kv_order):
                tb0 = tb * TBQ
                psk = [pkv.tile([128, TBQ], f32, name=f"psk_{tb}_{f}",
                                tag=f"pk{f}", bufs=2) for f in range(KVH)]
                psv = pkv.tile([128, KTB * FKV], f32, name=f"psv_{tb}",
                               tag="pv", bufs=1)
                for k in range(KH):
                    if ti == 0 and k == 24:
                        # sync queue is well ahead by now; must be emitted
                        # before block-1's K-rope reads (emission order = dep
                        # direction)
                        nc.sync.dma_start(cos_t[:], cosT.ap())
                        nc.sync.dma_start(sin_t[:], sinT.ap())
                    if ti == 2 and k == 16:
                        load_wq(2)
                    hs_s = load_hs(tb, k)
                    if ti == 0:
                        w = kvw.tile([128, 2 * FKV], mdt, name=f"wkv_{k}")
                        nc.scalar.dma_start(w[:], wkv.ap()[k * 128:(k + 1) * 128, :])
                        wkv_k.append(w)
                        if k == 0:
                            nc.scalar.dma_start(bq_t[:], bq.ap().rearrange("h p -> p h"))
                            nc.scalar.dma_start(bk_t[:], bk.ap().rearrange("h p -> p h"))
                            nc.scalar.dma_start(bvb_t[:], bvb.ap())
                            nc.scalar.dma_start(ones_t[:], ones.ap())
                    w = wkv_k[k]

                    def k_mm(f):
                        nc.tensor.matmul(psk[f][:], w[:, f * 128:(f + 1) * 128],
                                         hs_s[:], start=(k == 0), stop=(k == KH - 1))

                    def v_mm(tt):
                        # psv packs two 1KB accum slices per 2KB PSUM bank;
                        # start=True zeroes the whole bank (zero-region), so
                        # only the first slice of each bank may issue it.
                        nc.tensor.matmul(psv[:, tt * FKV:(tt + 1) * FKV],
                                         hs_s[:, tt * 128:(tt + 1) * 128],
                                         w[:, FKV:2 * FKV],
                                         start=(k == 0 and tt % 2 == 0),
                                         stop=(k == KH - 1))

                    # interleave so consecutive matmuls hit different banks
                    k_mm(0); v_mm(0); v_mm(2); k_mm(1); v_mm(1); v_mm(3)
                    # warmup: Q proj chunks for units 0/1 ride the last block
                    if ti == NSTEP - 1 and k % 4 == 3:
                        emit_q(0, 1)
                        emit_q(1, 1)
                # evacuations (v first: shortens the vector tail ahead of the
                # q-rope chain at the phase boundary) + K rope
                for tt in range(KTB):
                    kt = tb * KTB + tt
                    nc.vector.tensor_add(v_t[:, kt * FKV:(kt + 1) * FKV],
                                         psv[:, tt * FKV:(tt + 1) * FKV], bvb_t[:])
                for f in range(KVH):
                    nc.scalar.activation(k_t[f][:, tb0:tb0 + TBQ], psk[f][:], Ident,
                                         bias=bk_t[:, f:f + 1])
                for f in range(KVH):
                    rope_inplace(k_t[f], slice(tb0, tb0 + TBQ), f"k{f}_{tb}")
                if ti == 1:
                    load_wq(0)
                elif ti == 2:
                    load_wq(1)

        # ---- Phase 2: merged Q-proj + attention pipeline ----
        st3 = ctx.enter_context(tc.tile_pool(name="st3", bufs=4))  # wo prefetch
        wo_pre = []
        with (
            tc.tile_pool(name="expp", bufs=1) as expp,
            tc.tile_pool(name="spool", bufs=1) as spool,
            tc.tile_pool(name="invp", bufs=2) as invp,
            tc.tile_pool(name="pss", bufs=1, space="PSUM") as pss,
            tc.tile_pool(name="pso", bufs=1, space="PSUM") as pso,
        ):
            q_finish(0)
            q_finish(1)
            load_wq(3)

            for j, (tb, h) in enumerate(UNITS):
                sl = slice(tb * TBQ, (tb + 1) * TBQ)
                f = h // (QH // KVH)
                qj = j + 2 if j + 2 < len(UNITS) else None
                if j + 4 < len(UNITS):
                    load_wq(j + 4)
                # prefetch next step's hs tiles (8 per unit over h = 3..6)
                if 3 <= h <= 6 and tb + 1 < NSTEP:
                    for k in range((h - 3) * 8, (h - 3) * 8 + 8):
                        load_hs(tb + 1, k)
                if j == len(UNITS) - 3:
                    for fh in range(QH):
                        w = st3.tile([128, 512], mdt, name=f"wo_0_{fh}",
                                     tag=f"wo{fh}", bufs=2)
                        nc.sync.dma_start(w[:], wo.ap()[fh * 128:(fh + 1) * 128, 0:512])
                        wo_pre.append(w)

                po = pso.tile([128, TBQ], f32, name=f"po_{j}", tag="oo", bufs=2)
                ets = []

                def s_pair(p):
                    ps = pss.tile([128, 2 * TBQ], f32, name=f"ps_{j}_{p}",
                                  tag="ss", bufs=2)
                    for i2 in range(2):
                        kt = 2 * p + i2
                        nc.tensor.matmul(ps[:, i2 * TBQ:(i2 + 1) * TBQ],
                                         k_t[f][:, kt * 128:(kt + 1) * 128],
                                         q_t[h][:, sl], start=True, stop=True)
                    et = expp.tile([128, 2 * TBQ], mdt, name=f"e_{j}_{p}",
                                   tag="et", bufs=5)
                    nc.scalar.activation(et[:], ps[:], Exp, scale=SCALE)
                    ets.append(et)

                def a_pair(idx, p):
                    et = ets[idx]
                    for i2 in range(2):
                        kt = 2 * p + i2
                        nc.tensor.matmul(po[:],
                                         v_t[:, kt * FKV + f * 128:
                                             kt * FKV + (f + 1) * 128],
                                         et[:, i2 * TBQ:(i2 + 1) * TBQ],
                                         start=(idx == 0 and i2 == 0),
                                         stop=(idx == NPAIR - 1 and i2 == 1))

                # leading Q chunks have no dependency on this unit's rope, so
                # they lead the unit and absorb scalar/vector tail latency
                # process kt-pairs of the last-roped block (tb=0, pairs 0,1)
                # last so unit 0 never waits on the freshest K rope
                porder = [2, 3, 4, 5, 6, 7, 0, 1][:NPAIR]
                # last two units have no Q-proj left; fill their PE slack with
                # the first o-proj tiles (hb=0, tt=0..3) instead
                eo_tts = []
                if qj is None and NSTEP * QH - j <= 2:
                    eo_tts = [0, 1] if j == len(UNITS) - 2 else [2, 3]
                eo_pots = {}
                if qj is not None:
                    emit_q(qj, 5 if j == 0 else 1)
                s_pair(porder[0])
                s_pair(porder[1])
                t1s = []
                for i in range(NPAIR):
                    if qj is not None:
                        emit_q(qj, 1)
                    for tt in eo_tts:
                        if i == 0:
                            eo_pots[tt] = psqp.tile([128, 512], f32,
                                                    name=f"eo_{tt}", tag="psq",
                                                    bufs=2)
                        nc.tensor.matmul(eo_pots[tt][:],
                                         q_t[i][:, tt * 128:(tt + 1) * 128],
                                         wo_pre[i][:],
                                         start=(i == 0), stop=(i == QH - 1))
                    if i + 2 < NPAIR:
                        s_pair(porder[i + 2])
                    a_pair(i, porder[i])
                    if i % 2 == 1:
                        m = (i - 1) // 2
                        t1 = spool.tile([128, 2 * TBQ], mdt, name=f"t1_{j}_{m}",
                                        tag="pa", bufs=5)
                        nc.vector.tensor_add(t1[:], ets[i - 1][:], ets[i][:])
                        t1s.append(t1)
                u0 = spool.tile([128, 2 * TBQ], mdt, name=f"u0_{j}", tag="u2", bufs=2)
                nc.vector.tensor_add(u0[:], t1s[0][:], t1s[1][:])
                u1 = spool.tile([128, 2 * TBQ], mdt, name=f"u1_{j}", tag="u3", bufs=2)
                nc.vector.tensor_add(u1[:], t1s[2][:], t1s[3][:])
                nc.vector.tensor_add(u0[:], u0[:], u1[:])
                ssum = spool.tile([128, TBQ], mdt, name=f"sm_{j}", tag="sm", bufs=2)
                nc.vector.tensor_add(ssum[:], u0[:, 0:TBQ], u0[:, TBQ:2 * TBQ])
                pb = spool.tile([128, TBQ], f32, name=f"pb_{j}", tag="pbs", bufs=1)
                nc.gpsimd.partition_all_reduce(pb[:], ssum[:], 128,
                                               bass_isa.ReduceOp.add)
                inv = invp.tile([128, TBQ], f32, name=f"inv_{j}", tag="inv")
                nc.vector.reciprocal_approx_fast(inv[:], pb[:])
                # normalized attn output, overwrites q head block in place
                nc.vector.tensor_mul(q_t[h][:, sl], po[:], inv[:])
                for tt in eo_tts:
                    ot = invp.tile([128, 512], f32, name=f"eot_{tt}", tag="ot0",
                                   bufs=2)
                    nc.scalar.copy(ot[:], eo_pots[tt][:])
                    nc.sync.dma_start(out.ap()[tt * 128:(tt + 1) * 128, 0:512],
                                      ot[:])
                if qj is not None:
                    emit_q(qj, 8)
                    q_finish(qj)

        # ---- Phase 3: output projection (partial; host sums over TP) ----
        with (
            tc.tile_pool(name="osb", bufs=4) as osb,
            tc.tile_pool(name="po5", bufs=1, space="PSUM") as po5,
        ):
            NHB = HID // 512
            for hb in range(NHB):
                if hb == 0:
                    wo_s = wo_pre
                else:
                    wo_s = []
                    for fh in range(QH):
                        w = st3.tile([128, 512], mdt, name=f"wo_{hb}_{fh}",
                                     tag=f"wo{fh}", bufs=2)
                        nc.sync.dma_start(
                            w[:], wo.ap()[fh * 128:(fh + 1) * 128,
                                          hb * 512:(hb + 1) * 512])
                        wo_s.append(w)
                for tt in range(4 if hb == 0 else 0, KT):
                    pot = po5.tile([128, 512], f32, name=f"pot_{hb}_{tt}",
                                   tag="po", bufs=4)
                    for fh in range(QH):
                        nc.tensor.matmul(pot[:], q_t[fh][:, tt * 128:(tt + 1) * 128],
                                         wo_s[fh][:], start=(fh == 0),
                                         stop=(fh == QH - 1))
                    ot = osb.tile([128, 512], f32, name=f"ot_{hb}_{tt}", tag="ot")
                    nc.scalar.copy(ot[:], pot[:])
                    nc.sync.dma_start(
                        out.ap()[tt * 128:(tt + 1) * 128, hb * 512:(hb + 1) * 512],
                        ot[:])

    nc.compile()
    return nc


def shard_inputs(hidden_states, cos, sin, Wq, bq, Wk, bk, Wv, bv, Wo, S=S_FULL,
                 dt="bf16"):
    import ml_dtypes
    big = ml_dtypes.bfloat16
    ones = np.ones((128, 128), dtype=np.float32)
    in_maps = []
    for c in range(8):
        b, t = c // TP, c % TP
        sinT = np.ascontiguousarray(sin[b].T).astype(np.float32)
        sinT[:HD // 2, :] *= -1.0   # rotate_half sign folded into the table
        wq_slice = Wq[:, t * FQ:(t + 1) * FQ]                      # [HID, FQ]
        wqh = np.ascontiguousarray(
            wq_slice.reshape(HID, QH, HD).transpose(1, 0, 2).reshape(QH * HID, HD))
        bv_slice = bv[t * FKV:(t + 1) * FKV].astype(np.float32)
        m = {
            "hsT": np.ascontiguousarray(hidden_states[b].T).astype(big),
            "cosT": np.ascontiguousarray(cos[b].T).astype(big),
            "sinT": sinT.astype(big),
            "wqh": wqh.astype(big),
            "bq": np.ascontiguousarray(bq[t * FQ:(t + 1) * FQ].reshape(QH, HD)),
            "bk": np.ascontiguousarray(bk[t * FKV:(t + 1) * FKV].reshape(KVH, HD)),
            "bvb": np.ascontiguousarray(
                np.broadcast_to(bv_slice[None, :], (128, FKV))).astype(np.float32),
            "wkv": np.ascontiguousarray(np.concatenate(
                [Wk[:, t * FKV:(t + 1) * FKV], Wv[:, t * FKV:(t + 1) * FKV]],
                axis=1)).astype(big),
            "wo": np.ascontiguousarray(Wo[t * FQ:(t + 1) * FQ, :]).astype(big),
            "ones": ones.astype(big),
        }
        in_maps.append(m)
    return in_maps


_nc_cache = {}


def kernel(hidden_states, cos, sin, Wq, bq, Wk, bk, Wv, bv, Wo):
    global last_exec_time_ns
    from concourse.bass_utils import run_bass_kernel_spmd

    hidden_states = np.asarray(hidden_states, dtype=np.float32)
    cos = np.asarray(cos, dtype=np.float32)
    sin = np.asarray(sin, dtype=np.float32)
    S = hidden_states.shape[1]
    dt = "bf16"
    if (S, dt) not in _nc_cache:
        _nc_cache[(S, dt)] = build_nc(S, dt)
    nc = _nc_cache[(S, dt)]
    in_maps = shard_inputs(hidden_states, cos, sin,
                           np.asarray(Wq, np.float32), np.asarray(bq, np.float32),
                           np.asarray(Wk, np.float32), np.asarray(bk, np.float32),
                           np.asarray(Wv, np.float32), np.asarray(bv, np.float32),
                           np.asarray(Wo, np.float32), S=S, dt=dt)
    trace = bool(int(os.environ.get("ATTN_TRACE", "0")))
    r = run_bass_kernel_spmd(nc, in_maps, list(range(8)), trace=trace)
    last_exec_time_ns = r.exec_time_ns
    outs = [r.results[c]["out"] for c in range(8)]
    full = np.empty((B, S, HID), dtype=np.float32)
    for b in range(B):
        full[b] = outs[b * TP]
        for t in range(1, TP):
            full[b] += outs[b * TP + t]
    return full
